# revision 1
# baseline (speedup 1.0000x reference)
"""MemoryReader kernel for Trainium2, data-parallel over batch across 8 cores.

Per batch element b (one NeuronCore each):
    mkf = mk[b] as [CK=64, M=4096], qkf = qk[b] as [CK, N=4096]
    aff[m, n] = (2 * mkf.T @ qkf - |mkf[:,m]|^2) / sqrt(CK)
    P = softmax over m
    mem[c, n]  = sum_m mv[b][c, m] * P[m, n]
    out[b] = concat([mem, qv[b]], channel axis)

Device kernel layout (per core):
    - QK^T matmuls produce aff tiles in [m-partition, n-free] layout,
      32 m-chunks of [128, 512] per n-super-tile of 512 columns.
    - ScalarE computes E = exp(0.25*ab - a_sq/8) straight out of PSUM
      (per-partition bias = -a_sq/8; logits are bounded so the max
      subtraction of a standard softmax is unnecessary in fp32).
    - VectorE accumulates sum_m E chunk-by-chunk; a ones-vector matmul
      folds the partition axis; reciprocal + DMA partition-broadcast
      give 1/s replicated across partitions.
    - Readout matmuls contract over m in PSUM (4 c-chunks of 128), then
      VectorE scales by 1/s while evacuating PSUM.
    - mv^T / mk^T are prepared host-side (pure layout transforms), so no
      on-device transposes are needed. qv never touches the device.
"""

import os
import sys

import numpy as np

B, CK, CV, H, W = 8, 64, 512, 64, 64
M = H * W          # memory positions per batch element
N = H * W          # query positions
NT = 512           # n-super-tile width (columns per softmax pass)
NSUP = N // NT     # 8 n-super-tiles
MCH = M // 128     # 32 m-chunks
N_CORES = 8

# "fp32r" runs matmuls in relaxed-precision single-pass mode (4x faster
# than exact fp32 on the PE array); "fp32" is exact.
MATMUL_PREC = os.environ.get("KERNEL_MATMUL_PREC", "fp32r")

_CACHE = {}


def _build_program():
    sys.path.insert(0, "/opt/trn_rl_repo")
    from contextlib import ExitStack

    import concourse.tile as tile
    from concourse import bacc, mybir

    dt = mybir.dt
    f32 = dt.float32
    # Matmul operand dtype: float32r (relaxed single-pass fp32, 4x faster
    # on the PE array) or exact float32. Bit-layout is identical; walrus
    # requires producers of fp32r matmul operands to be typed fp32r.
    mdt = dt.float32r if MATMUL_PREC == "fp32r" else f32

    nc = bacc.Bacc("TRN2", target_bir_lowering=False, debug=False,
                   num_devices=N_CORES)

    mk_d = nc.dram_tensor("mk", [128, M], mdt, kind="ExternalInput").ap()
    mkt_d = nc.dram_tensor("mkt", [128, MCH * CK], f32,
                           kind="ExternalInput").ap()
    qk_d = nc.dram_tensor("qk", [128, N], mdt, kind="ExternalInput").ap()
    mvt_d = nc.dram_tensor("mvt", [MCH, 128, CV], mdt,
                           kind="ExternalInput").ap()
    mem_d = nc.dram_tensor("mem", [CV, N], f32, kind="ExternalOutput").ap()

    with tile.TileContext(nc) as tc, ExitStack() as ctx:
        sing = ctx.enter_context(tc.tile_pool(name="sing", bufs=1))
        e_pool = ctx.enter_context(tc.tile_pool(name="E", bufs=17))
        scratch = ctx.enter_context(tc.tile_pool(name="scratch", bufs=2))
        sacc_pool = ctx.enter_context(tc.tile_pool(name="sacc", bufs=2))
        row_pool = ctx.enter_context(tc.tile_pool(name="row", bufs=2))
        rb_pool = ctx.enter_context(tc.tile_pool(name="rb", bufs=2))
        out_pool = ctx.enter_context(tc.tile_pool(name="out", bufs=8))
        qk_ps_pool = ctx.enter_context(
            tc.tile_pool(name="qkps", bufs=2, space="PSUM"))
        ro_ps_pool = ctx.enter_context(
            tc.tile_pool(name="rops", bufs=1, space="PSUM"))


        # PE warmup: the PE activity monitor starts throttled at 1.2 GHz
        # and needs ~3.4us of sustained matmul activity to unthrottle.
        # Burn dummy matmuls while the input DMAs stream so the real
        # matmuls start at 2.4 GHz.
        warm_sb = sing.tile([128, NT], f32)
        nc.vector.memset(warm_sb[:], 1.0)
        warm_ps = qk_ps_pool.tile([128, NT], f32, tag="qk_ps", name="warm_ps")
        for w in range(56):
            nc.tensor.matmul(warm_ps[:, 0:128], lhsT=warm_sb[:, 0:128],
                             rhs=warm_sb[:, 0:128], start=True, stop=True)

        # Resident inputs. mk/qk are zero-padded from CK=64 to K=128
        # contraction rows: K=64 matmuls leave the PE activity monitor
        # throttled at 1.2 GHz (measured 427 ns/MM vs 222 ns at K=128),
        # so padded K=128 matmuls are 2x faster despite wasting rows.
        # All DMAs go through the sync engine (hardware DGE); ordered so
        # the tensors gating the first matmuls arrive first.
        mk_sb = sing.tile([128, M], mdt)
        qk_sb = sing.tile([128, N], mdt)
        mkt_sb = sing.tile([128, MCH, CK], f32)
        mvt_sb = sing.tile([128, MCH, CV], mdt)
        for g in range(4):
            gs = slice(g * 1024, (g + 1) * 1024)
            nc.sync.dma_start(out=mk_sb[:, gs], in_=mk_d[:, gs])
        nc.sync.dma_start(out=qk_sb[:, 0:NT], in_=qk_d[:, 0:NT])
        nc.sync.dma_start(out=mkt_sb[:], in_=mkt_d[:].rearrange(
            "p (j c) -> p j c", c=CK))
        for j in range(4):
            nc.sync.dma_start(out=mvt_sb[:, j, :], in_=mvt_d[j])
        nc.sync.dma_start(out=qk_sb[:, NT:N], in_=qk_d[:, NT:N])
        for j in range(4, MCH):
            nc.sync.dma_start(out=mvt_sb[:, j, :], in_=mvt_d[j])

        # Ones vectors typed fp32r so the softmax-sum and broadcast
        # matmuls take the single-pass PE path (213 ns vs 853 ns).
        ones_f32 = sing.tile([128, 1], f32)
        nc.vector.memset(ones_f32[:], 1.0)
        ones_sb = sing.tile([128, 1], mdt)
        nc.vector.tensor_copy(ones_sb[:], ones_f32[:].bitcast(mdt))
        ones_row_f32 = sing.tile([1, 128], f32)
        nc.vector.memset(ones_row_f32[:], 1.0)
        ones_row = sing.tile([1, 128], mdt)
        nc.vector.tensor_copy(ones_row[:], ones_row_f32[:].bitcast(mdt))

        # Per-partition softmax bias: asq[p, j] = -|mk[:, j*128+p]|^2 / 8.
        # (tensor_tensor_reduce crashes on HW via this toolchain; use
        # Square -> free-axis reduce -> scale, in 4 pieces to keep the
        # scratch small.)
        asq = sing.tile([128, MCH], f32)
        for piece in range(4):
            js = slice(piece * 8, (piece + 1) * 8)
            sqp = scratch.tile([128, 8, CK], f32, tag="sqp",
                               name=f"sqp{piece}")
            nc.scalar.activation(sqp[:], mkt_sb[:, js, :],
                                 mybir.ActivationFunctionType.Square)
            nc.vector.tensor_reduce(asq[:, js], sqp[:],
                                    axis=mybir.AxisListType.X,
                                    op=mybir.AluOpType.add)
        nc.scalar.mul(asq[:], asq[:], -0.125)
        # g[p, j] = exp(-|mk row|^2 / 8); folded into the value rows and
        # the denominator accumulation so the exp needs no bias and can
        # span two PSUM banks per instruction.
        g_col = sing.tile([128, MCH], f32)
        nc.scalar.activation(g_col[:], asq[:],
                             mybir.ActivationFunctionType.Exp)
        with nc.allow_low_precision(reason="fp32r is fp32 bits"):
            for j in range(MCH):
                nc.vector.tensor_scalar_mul(mvt_sb[:, j, :],
                                            mvt_sb[:, j, :],
                                            g_col[:, j:j + 1])

        def emit_tail(ti, tsacc, tosbs, tnsl):
            # Softmax denominator, reciprocal, partition-broadcast and
            # final scaling for super `ti`. Emitted a few chunks into the
            # NEXT super so the PE stream has QK matmuls to chew on while
            # the DVE-side reduction chain resolves.
            s_ps = qk_ps_pool.tile([1, NT], f32, tag="qk_ps",
                                   name=f"sps{ti}")
            nc.tensor.matmul(s_ps[:], lhsT=ones_sb[:], rhs=tsacc[:],
                             start=True, stop=True)
            s_row = row_pool.tile([1, NT], mdt, tag="srow",
                                  name=f"srow{ti}")
            with nc.allow_low_precision(reason="fp32r is fp32 bits"):
                nc.vector.reciprocal(s_row[:], s_ps[:].bitcast(mdt))
            rb_ps = qk_ps_pool.tile([128, NT], f32, tag="qk_ps",
                                    name=f"rbps{ti}")
            nc.tensor.matmul(rb_ps[:], lhsT=ones_row[:], rhs=s_row[:],
                             start=True, stop=True)
            rb = rb_pool.tile([128, NT], f32, tag="rb", name=f"rb{ti}")
            nc.scalar.copy(rb[:], rb_ps[:])
            for c in range(4):
                nc.vector.tensor_mul(tosbs[c][:], tosbs[c][:], rb[:])
                nc.sync.dma_start(
                    out=mem_d[c * 128:(c + 1) * 128, tnsl], in_=tosbs[c][:])

        pending_tail = None
        for i in range(NSUP):
            nsl = slice(i * NT, (i + 1) * NT)
            ro_ps = [ro_ps_pool.tile([128, NT], f32, tag=f"ro{c}",
                                     name=f"ro{c}_{i}")
                     for c in range(4)]
            sacc = sacc_pool.tile([128, NT], mdt, tag="sacc",
                                  name=f"sacc{i}")
            for t in range(MCH // 2):
                ma, mb = 2 * t, 2 * t + 1
                qk_ps = qk_ps_pool.tile([128, 2 * NT], f32, tag="qk_ps",
                                        name=f"qkps{i}_{t}")
                for h, m in ((0, ma), (1, mb)):
                    nc.tensor.matmul(
                        qk_ps[:, h * NT:(h + 1) * NT],
                        lhsT=mk_sb[:, m * 128:(m + 1) * 128],
                        rhs=qk_sb[:, nsl],
                        start=True, stop=True)
                e = e_pool.tile([128, 2 * NT], mdt, tag="E",
                                name=f"e{i}_{t}")
                nc.scalar.activation(
                    e[:], qk_ps[:], mybir.ActivationFunctionType.Exp,
                    scale=0.25)
                # sacc += g[m] * E chunk; fp32r is bit-identical to fp32,
                # the low-precision gate only keys off the dtype tag.
                with nc.allow_low_precision(reason="fp32r is fp32 bits"):
                    for h, m in ((0, ma), (1, mb)):
                        eh = e[:, h * NT:(h + 1) * NT]
                        if m == 0:
                            nc.vector.tensor_scalar_mul(
                                sacc[:], eh, g_col[:, m:m + 1])
                        else:
                            nc.vector.scalar_tensor_tensor(
                                out=sacc[:], in0=eh,
                                scalar=g_col[:, m:m + 1], in1=sacc[:],
                                op0=mybir.AluOpType.mult,
                                op1=mybir.AluOpType.add)
                if t == 2 and pending_tail is not None:
                    emit_tail(*pending_tail)
                    pending_tail = None
                for h, m in ((0, ma), (1, mb)):
                    for c in range(4):
                        nc.tensor.matmul(
                            ro_ps[c][:],
                            lhsT=mvt_sb[:, m, c * 128:(c + 1) * 128],
                            rhs=e[:, h * NT:(h + 1) * NT],
                            start=(m == 0), stop=(m == MCH - 1))

            # Evacuate readout PSUM unscaled right away so the next
            # n-super's readout matmuls get their banks back without
            # waiting on the softmax-sum/reciprocal chain.
            osbs = []
            for c in range(4):
                osb = out_pool.tile([128, NT], f32, tag="osb",
                                    name=f"osb{i}_{c}")
                nc.vector.tensor_copy(osb[:], ro_ps[c][:])
                osbs.append(osb)
            pending_tail = (i, sacc, osbs, nsl)

        emit_tail(*pending_tail)

    nc.compile()
    return nc


def _get_program():
    if "nc" not in _CACHE:
        _CACHE["nc"] = _build_program()
    return _CACHE["nc"]


def _make_in_maps(mk, qk, mv):
    mk = np.asarray(mk, dtype=np.float32)
    qk = np.asarray(qk, dtype=np.float32)
    mv = np.asarray(mv, dtype=np.float32)
    in_maps = []
    zpad = np.zeros((128 - CK, M), dtype=np.float32)
    for b in range(B):
        mk_b = np.ascontiguousarray(
            np.concatenate([mk[b].reshape(CK, M), zpad], axis=0))
        qk_b = np.ascontiguousarray(
            np.concatenate([qk[b].reshape(CK, N), zpad], axis=0))
        # mkt[p, j*CK + c] = mk[b][c, j*128 + p]
        mkt_b = np.ascontiguousarray(
            mk[b].reshape(CK, MCH, 128).transpose(2, 1, 0).reshape(
                128, MCH * CK))
        # mvt[j, p, c] = mv[b][c, j*128 + p]
        mvt_b = np.ascontiguousarray(
            mv[b].reshape(CV, MCH, 128).transpose(1, 2, 0))
        in_maps.append({"mk": mk_b, "qk": qk_b, "mkt": mkt_b, "mvt": mvt_b})
    return in_maps


def kernel(mk, qk, mv, qv):
    qv = np.asarray(qv, dtype=np.float32)
    nc = _get_program()
    from concourse.bass_utils import run_bass_kernel_spmd

    in_maps = _make_in_maps(mk, qk, mv)
    res = run_bass_kernel_spmd(nc, in_maps, list(range(N_CORES)))
    mem = np.stack([res.results[b]["mem"] for b in range(B)], axis=0)
    mem = mem.reshape(B, CV, H, W)
    return np.concatenate([mem, qv], axis=1)



# revision 2
# speedup vs baseline: 1.5445x; 1.5445x over previous
"""MemoryReader kernel for Trainium2, data-parallel over batch across 8 cores.

Per batch element b (one NeuronCore each):
    mkf = mk[b] as [CK=64, M=4096], qkf = qk[b] as [CK, N=4096]
    aff[m, n] = (2 * mkf.T @ qkf - |mkf[:,m]|^2) / sqrt(CK)
    P = softmax over m
    mem[c, n]  = sum_m mv[b][c, m] * P[m, n]
    out[b] = concat([mem, qv[b]], channel axis)

Device kernel layout (per core), v2 (fp8 DoubleRow readout):
    - QK^T matmuls produce logit chunks in [m-partition, n-free] layout,
      one [128, 512] PSUM bank per m-chunk, 3 rotating banks.
    - ScalarE computes E = exp(0.25*ab + bias) per chunk straight out of
      PSUM with per-partition bias = 2 - |mk col|^2/8 (host-computed),
      writing float8e4 halves of a [128, 2, 512] pair tile. The +2 shift
      centers E in fp8 range; it cancels between numerator/denominator.
    - Softmax denominator: a DoubleRow ones-matmul accumulates
      s[n] = sum_m E[m, n] in a [1, 512] PSUM bank (16 pair-MMs/super).
    - Readout matmuls contract over m in fp8 DoubleRow mode (256 rows
      per instruction): 16 pair-MMs x 4 c-chunks per n-super.
    - PSUM budget: 3 (qk) + 4 (readout) + 1 (denominator) = 8 banks.
    - Tail per super: ScalarE evacuates s to SBUF, DVE reciprocal,
      ones-row matmul broadcasts 1/s across partitions, DVE scales the
      evacuated readout and DMAs out. Emitted a few pairs into the next
      super so PE never waits on the DVE chain.
    - mv^T is quantized to fp8e4 and laid out host-side; asq bias is
      host-computed. qv never touches the device.
"""

import os
import sys

import numpy as np

B, CK, CV, H, W = 8, 64, 512, 64, 64
M = H * W          # memory positions per batch element
N = H * W          # query positions
NT = 512           # n-super-tile width (columns per softmax pass)
NSUP = N // NT     # 8 n-super-tiles
MCH = M // 128     # 32 m-chunks
PAIRS = MCH // 2   # 16 m-chunk pairs (DoubleRow contracts 256 rows)
N_CORES = 8

_CACHE = {}


def _build_program():
    sys.path.insert(0, "/opt/trn_rl_repo")
    from contextlib import ExitStack

    import concourse.tile as tile
    from concourse import bacc, mybir

    dt = mybir.dt
    f32 = dt.float32
    f32r = dt.float32r
    f8 = dt.float8e4
    DR = mybir.MatmulPerfMode.DoubleRow

    nc = bacc.Bacc("TRN2", target_bir_lowering=False, debug=False,
                   num_devices=N_CORES)

    mk_d = nc.dram_tensor("mk", [128, M], f32r, kind="ExternalInput").ap()
    qk_d = nc.dram_tensor("qk", [128, N], f32r, kind="ExternalInput").ap()
    mvt_d = nc.dram_tensor("mvt", [128, MCH * CV], f8,
                           kind="ExternalInput").ap()
    asq_d = nc.dram_tensor("asq", [128, MCH], f32,
                           kind="ExternalInput").ap()
    mem_d = nc.dram_tensor("mem", [CV, N], f32, kind="ExternalOutput").ap()

    with tile.TileContext(nc) as tc, ExitStack() as ctx:
        sing = ctx.enter_context(tc.tile_pool(name="sing", bufs=1))
        e_pool = ctx.enter_context(tc.tile_pool(name="E", bufs=6))
        row_pool = ctx.enter_context(tc.tile_pool(name="row", bufs=2))
        rb_pool = ctx.enter_context(tc.tile_pool(name="rb", bufs=2))
        out_pool = ctx.enter_context(tc.tile_pool(name="out", bufs=8))
        qk_ps_pool = ctx.enter_context(
            tc.tile_pool(name="qkps", bufs=3, space="PSUM"))
        ro_ps_pool = ctx.enter_context(
            tc.tile_pool(name="rops", bufs=1, space="PSUM"))
        s_ps_pool = ctx.enter_context(
            tc.tile_pool(name="sps", bufs=1, space="PSUM"))

        # PE warmup: the PE activity monitor starts throttled at 1.2 GHz
        # and needs ~3.4us of sustained matmul activity to unthrottle.
        # Burn dummy matmuls while the input DMAs stream so the real
        # matmuls start at 2.4 GHz.
        warm_sb = sing.tile([128, 128], f32)
        nc.vector.memset(warm_sb[:], 1.0)
        warm_ps = qk_ps_pool.tile([128, NT], f32, tag="qk_ps", name="warm_ps")
        for w in range(56):
            nc.tensor.matmul(warm_ps[:, 0:128], lhsT=warm_sb[:],
                             rhs=warm_sb[:], start=True, stop=True)

        # Resident inputs. mk/qk are zero-padded from CK=64 to K=128
        # contraction rows: K=64 matmuls leave the PE activity monitor
        # throttled at 1.2 GHz, so padded K=128 matmuls are 2x faster
        # despite wasting rows. DMAs ordered so the tensors gating the
        # first matmuls arrive first.
        asq_sb = sing.tile([128, MCH], f32)
        mk_sb = sing.tile([128, M], f32r)
        qk_sb = sing.tile([128, N], f32r)
        mvt_sb = sing.tile([128, MCH, CV], f8)
        nc.sync.dma_start(out=asq_sb[:], in_=asq_d[:])
        nc.sync.dma_start(out=qk_sb[:, 0:NT], in_=qk_d[:, 0:NT])
        for g in range(4):
            gs = slice(g * 1024, (g + 1) * 1024)
            nc.sync.dma_start(out=mk_sb[:, gs], in_=mk_d[:, gs])
        for g in range(4):
            gs = slice(g * 8 * CV, (g + 1) * 8 * CV)
            nc.sync.dma_start(
                out=mvt_sb[:, g * 8:(g + 1) * 8, :],
                in_=mvt_d[:, gs].rearrange("p (j c) -> p j c", c=CV))
        nc.sync.dma_start(out=qk_sb[:, NT:N], in_=qk_d[:, NT:N])

        # Ones operands: fp8 pair-column for the DoubleRow denominator
        # matmul, fp32r row for the 1/s partition-broadcast matmul.
        ones_f32 = sing.tile([128, 2, 16], f32)
        nc.vector.memset(ones_f32[:], 1.0)
        ones2 = sing.tile([128, 2, 16], f8)
        with nc.allow_low_precision(reason="exact value 1.0 in fp8"):
            nc.vector.tensor_copy(ones2[:], ones_f32[:])
        ones_row_f32 = sing.tile([1, 128], f32)
        nc.vector.memset(ones_row_f32[:], 1.0)
        ones_row = sing.tile([1, 128], f32r)
        nc.vector.tensor_copy(ones_row[:], ones_row_f32[:].bitcast(f32r))

        def emit_tail(ti, ts_sb, tosbs, tnsl):
            # 1/s chain and final scaling for super `ti`, emitted a few
            # pairs into the NEXT super so the PE stream keeps running
            # while the DVE-side chain resolves.
            s_row = row_pool.tile([1, NT], f32r, tag="srow",
                                  name=f"srow{ti}")
            with nc.allow_low_precision(reason="fp32r is fp32 bits"):
                nc.vector.reciprocal(s_row[:], ts_sb[:].bitcast(f32r))
            rb_ps = qk_ps_pool.tile([128, NT], f32, tag="qk_ps",
                                    name=f"rbps{ti}")
            nc.tensor.matmul(rb_ps[:], lhsT=ones_row[:], rhs=s_row[:],
                             start=True, stop=True)
            rb = rb_pool.tile([128, NT], f32, tag="rb", name=f"rb{ti}")
            nc.scalar.copy(rb[:], rb_ps[:])
            for c in range(4):
                nc.vector.tensor_mul(tosbs[c][:], tosbs[c][:], rb[:])
                nc.sync.dma_start(
                    out=mem_d[c * 128:(c + 1) * 128, tnsl], in_=tosbs[c][:])

        pending_tail = None
        for i in range(NSUP):
            nsl = slice(i * NT, (i + 1) * NT)
            ro_ps = [ro_ps_pool.tile([128, NT], f32, tag=f"ro{c}",
                                     name=f"ro{c}_{i}")
                     for c in range(4)]
            s_ps = s_ps_pool.tile([1, NT], f32, tag="s_ps", name=f"sps{i}")
            for t in range(PAIRS):
                e = e_pool.tile([128, 2, NT], f8, tag="E", name=f"e{i}_{t}")
                for h, m in ((0, 2 * t), (1, 2 * t + 1)):
                    qk_ps = qk_ps_pool.tile([128, NT], f32, tag="qk_ps",
                                            name=f"qkps{i}_{t}_{h}")
                    nc.tensor.matmul(
                        qk_ps[:],
                        lhsT=mk_sb[:, m * 128:(m + 1) * 128],
                        rhs=qk_sb[:, nsl],
                        start=True, stop=True)
                    with nc.allow_low_precision(reason="fp8 softmax "
                                                "weights, tol 2e-2"):
                        nc.scalar.activation(
                            e[:, h, :], qk_ps[:],
                            mybir.ActivationFunctionType.Exp,
                            bias=asq_sb[:, m:m + 1], scale=0.25)
                nc.tensor.matmul(s_ps[:], lhsT=ones2[:, :, 0:1], rhs=e[:],
                                 perf_mode=DR,
                                 start=(t == 0), stop=(t == PAIRS - 1))
                if t == 2 and pending_tail is not None:
                    emit_tail(*pending_tail)
                    pending_tail = None
                for c in range(4):
                    nc.tensor.matmul(
                        ro_ps[c][:],
                        lhsT=mvt_sb[:, 2 * t:2 * t + 2,
                                    c * 128:(c + 1) * 128],
                        rhs=e[:],
                        perf_mode=DR,
                        start=(t == 0), stop=(t == PAIRS - 1))

            # Evacuate readout PSUM unscaled right away (split across
            # ScalarE and VectorE) so the next n-super's readout matmuls
            # get their banks back quickly; evacuate the denominator to
            # SBUF with ScalarE so its bank frees without waiting on the
            # DVE reciprocal chain.
            s_sb = row_pool.tile([1, NT], f32, tag="ssb", name=f"ssb{i}")
            nc.scalar.copy(s_sb[:], s_ps[:])
            osbs = []
            for c in range(4):
                osb = out_pool.tile([128, NT], f32, tag="osb",
                                    name=f"osb{i}_{c}")
                if c < 2:
                    nc.scalar.copy(osb[:], ro_ps[c][:])
                else:
                    nc.vector.tensor_copy(osb[:], ro_ps[c][:])
                osbs.append(osb)
            pending_tail = (i, s_sb, osbs, nsl)

        emit_tail(*pending_tail)

    nc.compile()
    return nc


def _get_program():
    if "nc" not in _CACHE:
        _CACHE["nc"] = _build_program()
    return _CACHE["nc"]


def _make_in_maps(mk, qk, mv):
    import ml_dtypes

    mk = np.asarray(mk, dtype=np.float32)
    qk = np.asarray(qk, dtype=np.float32)
    mv = np.asarray(mv, dtype=np.float32)
    in_maps = []
    zpad = np.zeros((128 - CK, M), dtype=np.float32)
    for b in range(B):
        mkf = mk[b].reshape(CK, M)
        mk_b = np.ascontiguousarray(np.concatenate([mkf, zpad], axis=0))
        qk_b = np.ascontiguousarray(
            np.concatenate([qk[b].reshape(CK, N), zpad], axis=0))
        # asq[p, j] = 2 - |mk[b][:, j*128+p]|^2 / 8  (exp bias; the +2
        # shift centers fp8 E and cancels against the denominator)
        asq_b = np.ascontiguousarray(
            (2.0 - (mkf * mkf).sum(axis=0) / 8.0)
            .reshape(MCH, 128).T.astype(np.float32))
        # mvt[p, j*CV + c] = mv[b][c, j*128 + p], quantized to fp8e4
        mvt_b = np.ascontiguousarray(
            mv[b].reshape(CV, MCH, 128).transpose(2, 1, 0)
            .reshape(128, MCH * CV).astype(ml_dtypes.float8_e4m3))
        in_maps.append({"mk": mk_b, "qk": qk_b, "mvt": mvt_b, "asq": asq_b})
    return in_maps


def kernel(mk, qk, mv, qv):
    qv = np.asarray(qv, dtype=np.float32)
    nc = _get_program()
    from concourse.bass_utils import run_bass_kernel_spmd

    in_maps = _make_in_maps(mk, qk, mv)
    res = run_bass_kernel_spmd(nc, in_maps, list(range(N_CORES)))
    mem = np.stack([res.results[b]["mem"] for b in range(B)], axis=0)
    mem = mem.reshape(B, CV, H, W)
    return np.concatenate([mem, qv], axis=1)


# revision 8
# speedup vs baseline: 1.8132x; 1.1740x over previous
"""MemoryReader kernel for Trainium2, data-parallel over batch across 8 cores.

Per batch element b (one NeuronCore each):
    mkf = mk[b] as [CK=64, M=4096], qkf = qk[b] as [CK, N=4096]
    aff[m, n] = (2 * mkf.T @ qkf - |mkf[:,m]|^2) / sqrt(CK)
    P = softmax over m
    mem[c, n]  = sum_m mv[b][c, m] * P[m, n]
    out[b] = concat([mem, qv[b]], channel axis)

Device kernel layout (per core), v3 (transposed fp8 DoubleRow readout):
    - QK^T matmuls produce logit chunks in [m-partition, n-free] layout,
      one [128, 512] PSUM bank per m-chunk, 3 rotating banks. The exp
      bias (2 - |mk col|^2/8, the +2 centers fp8 range and cancels in
      the softmax) rides in contraction row 64: mk row 64 holds
      8 - |col|^2/2 and qk row 64 holds 1.0, so the bias comes out of
      the matmul for free and ScalarE's exp needs no bias operand.
    - ScalarE computes E = exp(0.25 * psum) per chunk straight out of
      PSUM, writing float8e4 halves of a [128, 2, 512] pair tile.
    - Readout is TRANSPOSED: for each n-chunk k, out[n, c] accumulates
      lhsT = E-pair[:, :, 128k:128k+128] (stationary) against
      rhs = mv-pair [128, 2, 512] (moving) in fp8 DoubleRow mode
      (256 contraction rows per instruction). Output [n-part, c-free]
      makes the softmax 1/s a per-partition scalar.
    - Denominator: a DoubleRow ones-matmul accumulates s[n] per super in
      one [1, 512] PSUM row; rows 0/32 of one bank alternate between
      supers so the next super never waits on the tail chain.
    - PSUM budget: 3 (qk) + 4 (readout) + 1 (denominator) = 8 banks.
    - Tail per super (emitted a few pairs into the next super): ScalarE
      copies s to SBUF, four K=1 matmuls transpose it to [128, 4], DVE
      reciprocal + per-partition scale of the evacuated readout, DMA out.
    - mv^T is quantized to fp8e4 and laid out host-side. qv never
      touches the device. Output is [N, CV]; host transposes back.
"""

import os
import sys

import numpy as np

B, CK, CV, H, W = 8, 64, 512, 64, 64
M = H * W          # memory positions per batch element
N = H * W          # query positions
NT = 512           # n-super-tile width (columns per softmax pass)
NSUP = N // NT     # 8 n-super-tiles
MCH = M // 128     # 32 m-chunks
PAIRS = MCH // 2   # 16 m-chunk pairs (DoubleRow contracts 256 rows)
N_CORES = 8

_CACHE = {}


def _build_program():
    sys.path.insert(0, "/opt/trn_rl_repo")
    from contextlib import ExitStack

    import concourse.tile as tile
    from concourse import bacc, mybir

    dt = mybir.dt
    f32 = dt.float32
    f32r = dt.float32r
    f8 = dt.float8e4
    DR = mybir.MatmulPerfMode.DoubleRow

    nc = bacc.Bacc("TRN2", target_bir_lowering=False, debug=False,
                   num_devices=N_CORES)

    mk_d = nc.dram_tensor("mk", [128, M], f32r, kind="ExternalInput").ap()
    qk_d = nc.dram_tensor("qk", [128, N], f32r, kind="ExternalInput").ap()
    mvt_d = nc.dram_tensor("mvt", [128, MCH * CV], f8,
                           kind="ExternalInput").ap()
    mem_d = nc.dram_tensor("mem", [N, CV], f32, kind="ExternalOutput").ap()

    with tile.TileContext(nc) as tc, ExitStack() as ctx:
        sing = ctx.enter_context(tc.tile_pool(name="sing", bufs=1))
        e_pool = ctx.enter_context(tc.tile_pool(name="E", bufs=8))
        row_pool = ctx.enter_context(tc.tile_pool(name="row", bufs=2))
        inv_pool = ctx.enter_context(tc.tile_pool(name="inv", bufs=2))
        out_pool = ctx.enter_context(tc.tile_pool(name="out", bufs=8))
        qk_ps_pool = ctx.enter_context(
            tc.tile_pool(name="qkps", bufs=3, space="PSUM"))
        ro_ps_pool = ctx.enter_context(
            tc.tile_pool(name="rops", bufs=1, space="PSUM"))
        s_ps_pool = ctx.enter_context(
            tc.tile_pool(name="sps", bufs=1, space="PSUM"))

        # PE warmup: the PE activity monitor starts throttled at 1.2 GHz
        # and needs ~3.4us of sustained matmul activity to unthrottle.
        # Burn dummy matmuls while the input DMAs stream so the real
        # matmuls start at 2.4 GHz.
        warm_sb = sing.tile([128, 128], f32)
        nc.vector.memset(warm_sb[:], 1.0)
        warm_ps = qk_ps_pool.tile([128, NT], f32, tag="qk_ps", name="warm_ps")
        for w in range(32):
            nc.tensor.matmul(warm_ps[:, 0:128], lhsT=warm_sb[:],
                             rhs=warm_sb[:], start=True, stop=True)

        # Resident inputs. mk/qk carry CK=64 data rows, the exp-bias/ones
        # row at 64, and zero padding to K=128 (K=64 matmuls run at the
        # throttled PE clock, so padded K=128 is 2x faster). DMAs ordered
        # so the tensors gating the first matmuls arrive first.
        mk_sb = sing.tile([128, M], f32r)
        qk_sb = sing.tile([128, N], f32r)
        mvt_sb = sing.tile([128, MCH, CV], f8)
        nc.sync.dma_start(out=qk_sb[:, 0:NT], in_=qk_d[:, 0:NT])
        for g in range(4):
            gs = slice(g * 1024, (g + 1) * 1024)
            nc.sync.dma_start(out=mk_sb[:, gs], in_=mk_d[:, gs])
        for g in range(4):
            gs = slice(g * 8 * CV, (g + 1) * 8 * CV)
            nc.sync.dma_start(
                out=mvt_sb[:, g * 8:(g + 1) * 8, :],
                in_=mvt_d[:, gs].rearrange("p (j c) -> p j c", c=CV))
        nc.sync.dma_start(out=qk_sb[:, NT:N], in_=qk_d[:, NT:N])

        # Ones operands: fp8 pair-column for the DoubleRow denominator
        # matmul, fp32r single element for the s-transpose matmuls.
        ones_f32 = sing.tile([128, 2, 16], f32)
        nc.vector.memset(ones_f32[:], 1.0)
        ones2 = sing.tile([128, 2, 16], f8)
        with nc.allow_low_precision(reason="exact value 1.0 in fp8"):
            nc.vector.tensor_copy(ones2[:], ones_f32[:])
        one1 = sing.tile([1, 1], f32)
        nc.vector.memset(one1[:], 1.0)

        s_ps = s_ps_pool.tile([1, NT], f32, tag="s_ps", name="s_ps")

        def emit_tail(ti, ts_sb, tosbs, tnsl):
            # 1/s chain and final scaling for super `ti`, emitted a few
            # pairs into the NEXT super so the boundary engines stay
            # clear while the chain resolves.
            st = qk_ps_pool.tile([128, 4], f32, tag="qk_ps",
                                 name=f"st{ti}")
            for k in range(4):
                nc.tensor.matmul(
                    st[:, k:k + 1],
                    lhsT=ts_sb[0:1, k * 128:(k + 1) * 128],
                    rhs=one1[:], start=True, stop=True)
            inv_s = inv_pool.tile([128, 4], f32, tag="inv",
                                  name=f"inv{ti}")
            nc.vector.reciprocal(inv_s[:], st[:])
            with nc.allow_low_precision(reason="fp32 scale of fp32 data"):
                for k in range(4):
                    nc.vector.tensor_scalar_mul(
                        tosbs[k][:], tosbs[k][:], inv_s[:, k:k + 1])
            for k in range(4):
                nc.sync.dma_start(
                    out=mem_d[tnsl.start + k * 128:
                              tnsl.start + (k + 1) * 128, :],
                    in_=tosbs[k][:])

        def emit_evacs(pi, pro_ps):
            # Evacuate the previous super's readout PSUM unscaled (k=0-2
            # on the boundary-idle DVE, k=3 on ScalarE behind the new
            # super's first exps) plus the denominator row, so the banks
            # free without waiting on the 1/s chain.
            s_sb = row_pool.tile([1, NT], f32, tag="ssb", name=f"ssb{pi}")
            nc.vector.tensor_copy(s_sb[:], s_ps[:])
            osbs = []
            for k in range(4):
                osb = out_pool.tile([128, CV], f32, tag="osb",
                                    name=f"osb{pi}_{k}")
                if k < 3:
                    nc.vector.tensor_copy(osb[:], pro_ps[k][:])
                else:
                    nc.scalar.copy(osb[:], pro_ps[k][:])
                osbs.append(osb)
            return s_sb, osbs

        pending_tail = None
        prev = None
        for i in range(NSUP):
            nsl = slice(i * NT, (i + 1) * NT)
            ro_ps = None
            for t in range(PAIRS):
                e = e_pool.tile([128, 2, NT], f8, tag="E", name=f"e{i}_{t}")
                for h, m in ((0, 2 * t), (1, 2 * t + 1)):
                    qk_ps = qk_ps_pool.tile([128, NT], f32, tag="qk_ps",
                                            name=f"qkps{i}_{t}_{h}")
                    nc.tensor.matmul(
                        qk_ps[:],
                        lhsT=mk_sb[:, m * 128:(m + 1) * 128],
                        rhs=qk_sb[:, nsl],
                        start=True, stop=True)
                    with nc.allow_low_precision(reason="fp8 softmax "
                                                "weights, tol 2e-2"):
                        nc.scalar.activation(
                            e[:, h, :], qk_ps[:],
                            mybir.ActivationFunctionType.Exp, scale=0.25)
                if t == 0:
                    # Boundary: previous super's evacuations go out after
                    # this super's first exps so ScalarE's exp stream is
                    # not delayed; only then allocate this super's
                    # readout banks (pool read-before-realloc order).
                    if prev is not None:
                        ps_sb, posbs = emit_evacs(prev[0], prev[1])
                        pending_tail = (prev[0], ps_sb, posbs, prev[2])
                    ro_ps = [ro_ps_pool.tile([128, CV], f32, tag=f"ro{k}",
                                             name=f"ro{k}_{i}")
                             for k in range(4)]
                nc.tensor.matmul(s_ps[:], lhsT=ones2[:, :, 0:1],
                                 rhs=e[:], perf_mode=DR,
                                 start=(t == 0), stop=(t == PAIRS - 1))
                if t == 3 and pending_tail is not None:
                    emit_tail(*pending_tail)
                    pending_tail = None
                for k in range(4):
                    nc.tensor.matmul(
                        ro_ps[k][:],
                        lhsT=e[:, :, k * 128:(k + 1) * 128],
                        rhs=mvt_sb[:, 2 * t:2 * t + 2, :],
                        perf_mode=DR,
                        start=(t == 0), stop=(t == PAIRS - 1))
            prev = (i, ro_ps, nsl)

        s_sb, osbs = emit_evacs(prev[0], prev[1])
        emit_tail(prev[0], s_sb, osbs, prev[2])

    nc.compile()
    return nc


def _get_program():
    if "nc" not in _CACHE:
        _CACHE["nc"] = _build_program()
    return _CACHE["nc"]


def _make_in_maps(mk, qk, mv):
    import ml_dtypes

    mk = np.asarray(mk, dtype=np.float32)
    qk = np.asarray(qk, dtype=np.float32)
    mv = np.asarray(mv, dtype=np.float32)
    in_maps = []
    zpad = np.zeros((127 - CK, M), dtype=np.float32)
    ones_row = np.ones((1, N), dtype=np.float32)
    for b in range(B):
        mkf = mk[b].reshape(CK, M)
        # row 64 = 4 * (2 - |col|^2/8): exp-bias delivered via the matmul
        # (exp applies scale 0.25 afterwards)
        bias_row = (8.0 - 0.5 * (mkf * mkf).sum(axis=0))[None, :]
        mk_b = np.ascontiguousarray(
            np.concatenate([mkf, bias_row, zpad], axis=0))
        qk_b = np.ascontiguousarray(
            np.concatenate([qk[b].reshape(CK, N), ones_row, zpad], axis=0))
        # mvt[p, j*CV + c] = mv[b][c, j*128 + p], quantized to fp8e4
        mvt_b = np.ascontiguousarray(
            mv[b].reshape(CV, MCH, 128).transpose(2, 1, 0)
            .reshape(128, MCH * CV).astype(ml_dtypes.float8_e4m3))
        in_maps.append({"mk": mk_b, "qk": qk_b, "mvt": mvt_b})
    return in_maps


def kernel(mk, qk, mv, qv):
    qv = np.asarray(qv, dtype=np.float32)
    nc = _get_program()
    from concourse.bass_utils import run_bass_kernel_spmd

    in_maps = _make_in_maps(mk, qk, mv)
    res = run_bass_kernel_spmd(nc, in_maps, list(range(N_CORES)))
    mem = np.stack([res.results[b]["mem"].T for b in range(B)], axis=0)
    mem = np.ascontiguousarray(mem).reshape(B, CV, H, W)
    return np.concatenate([mem, qv], axis=1)
